# revision 15
# baseline (speedup 1.0000x reference)
"""Trainium2 Bass kernel for nn_CAGKE_1 (Gaussian-kernel embedding).

Math: reference computes, for mask m_i = 1[X_i > 0.5],
    out[j] = sum_e softmax(w)_e * sum_i m_i * (c/sigma_e) exp(-(j-i-1)^2/(2 sigma_e^2)) + noise_j
Both sums are linear, so the E=128 Gaussian channels collapse into one
combined kernel ghat(d) = sum_e softmax(w)_e * (c/sigma_e) exp(-d^2/(2 sigma_e^2))
BEFORE the convolution. sigma_max = 5 makes ghat(d) < 1e-9 for |d| >= 32,
so 64 taps (d = 31-u, u in [0,64)) cover it to far below the accuracy gate.

Layout strategy (per core, 1024 outputs):
  - The conv is ONE matmul shape: out[n] = sum_u gs[u] * M[u, n] with
    M[u, n] = m[base + u + n].  M is materialized by a single DMA whose
    DRAM access pattern has stride 1 in BOTH dims over a 1088-byte padded
    mask window - the 64x Toeplitz expansion happens inside the DMA
    (64 descriptors of 1KB), so the host ships 1.1KB instead of 144KB.
  - sigma/w arrive as ONE 256-wide f32 row on partition 0 (1 descriptor);
    the whole softmax/sigma chain runs in row space on partition 0, then
    two tiny PE transposes (moving = const [1,1] ones) lift
    t1 = c*exp(w)/(sigma*Z) and ivs = +1/(2 sigma^2) into column space.
    Z comes free from the Exp activation's accum_out, so there is no
    fp32 ones-matmul anywhere.  The exp table gets a NEGATED d^2 so ivs
    needs no sign flip.
  - The output lives on PSUM partitions {0,32,64,96} x 256 cols: four
    col-tiled matmuls (tile_position=(0,32k)) with the same 1-column gs
    stationary run CONCURRENTLY on the PE sub-arrays, each streaming a
    shifted 256-col slice of M.  A sparse-stationary matmul (S4[k,32k]=1)
    pre-loads the noise into the same PSUM region (start=True), so the
    epilogue is one DVE copy + a 4-descriptor store.
  - The X-0.5 shift is scaled by 64 on the host (pure affine + fp8 cast,
    as in the reference's binarize-vs-0.5) so the sign survives fp8;
    the device binarizes with is_gt 0.
"""

import sys

import numpy as np

if "/opt/trn_rl_repo" not in sys.path:
    sys.path.insert(0, "/opt/trn_rl_repo")

T = 8192
E = 128
N_CORES = 8
TJ = T // N_CORES          # 1024 outputs per core
NTAP = 64                  # ghat taps: d = 31 - u, u in [0, 64)
WIN = NTAP + TJ            # 1088-byte padded mask window per core
INV_SQRT_2PI = 0.39894228

_compiled = None


def _build():
    import concourse.bacc as bacc
    import concourse.bass as bass
    import concourse.mybir as mybir
    import concourse.tile as tile
    from concourse.ap import AP

    f32 = mybir.dt.float32
    bf16 = mybir.dt.bfloat16
    fp8 = mybir.dt.float8e4
    nc = bacc.Bacc(num_devices=N_CORES, debug=False)

    crit_d = nc.dram_tensor("crit", [1, 256], f32, kind="ExternalInput")
    maskr_d = nc.dram_tensor("maskr", [1, WIN], fp8, kind="ExternalInput")
    nm_d = nc.dram_tensor("nm", [4, 256], bf16, kind="ExternalInput")
    out_d = nc.dram_tensor("out", [4, 256], f32, kind="ExternalOutput")

    with tile.TileContext(nc) as tc:
        with (
            tc.tile_pool(name="pool", bufs=1) as pool,
            tc.tile_pool(name="psum", bufs=1, space="PSUM") as psum,
        ):
            # ---- three HWDGE loads; critical (sigma/w) first ----
            crit = pool.tile([1, 256], f32, tag="crit")
            nc.sync.dma_start(crit[:], crit_d[:])
            # Toeplitz-expanding mask load: partition u reads bytes
            # [u, u+1024) of the 1088-byte window (overlapping reads).
            mraw = pool.tile([NTAP, TJ], fp8, tag="mraw")
            mask_src = AP(maskr_d[:].tensor, 0, [(1, NTAP), (1, TJ)])
            nc.sync.dma_start(mraw[:], mask_src)
            # noise straight to SBUF partitions {0,32,64,96} (4 descriptors);
            # dead partitions are zeroed early by GpSimd so the final
            # full-partition add reads defined data
            noiseS = pool.tile([128, 256], bf16, tag="noiseS")
            nc.gpsimd.memset(noiseS[:], 0.0)
            noise_dst = AP(
                noiseS[:].tensor, noiseS[:].offset, [(32 * 256, 4), (1, 256)]
            )
            nc.sync.dma_start(noise_dst, nm_d[:])

            wrow = crit[0:1, 0:128]
            srow = crit[0:1, 128:256]
            onef = nc.const_aps.tensor(1.0, (1, 1), f32)
            oneb = nc.const_aps.tensor(1.0, (1, 1), bf16)

            # ---- input-independent prep (Pool engine, off-path) ----
            # d2n[., u] = -(u-31)^2 via (u-31)*(31-u)
            dlt = pool.tile([128, NTAP], f32, tag="dlt")
            nc.gpsimd.iota(
                dlt[:], pattern=[[1, NTAP]], base=-31, channel_multiplier=0,
                allow_small_or_imprecise_dtypes=True,
            )
            dltn = pool.tile([128, NTAP], f32, tag="dltn")
            nc.gpsimd.iota(
                dltn[:], pattern=[[-1, NTAP]], base=31, channel_multiplier=0,
                allow_small_or_imprecise_dtypes=True,
            )
            d2n = pool.tile([128, NTAP], f32, tag="d2n")
            nc.gpsimd.tensor_mul(d2n[:], dlt[:], dltn[:])
            onesr = pool.tile([1, 128], bf16, tag="onesr")
            nc.gpsimd.memset(onesr[:], 1.0)
            zrow = pool.tile([1, 256], bf16, tag="zrow")
            nc.gpsimd.memset(zrow[:], 0.0)

            # ---- transpose sigma early: row -> column (PE, f32) ----
            pSc = psum.tile([128, 1], f32, tag="pSc")
            nc.tensor.matmul(pSc[:], srow, onef, is_transpose=True)
            # softmax exp in row space; accum_out gives Z for free
            ew = pool.tile([1, 132], f32, tag="ew")
            nc.scalar.activation(
                ew[:, 0:128], wrow, mybir.ActivationFunctionType.Exp,
                accum_out=ew[:, 128:129],
            )

            # ---- DVE chain, critical ops first (queue is program-order) ----
            sc = pool.tile([128, 1], f32, tag="sc")
            nc.vector.tensor_copy(sc[:], pSc[:])
            s2c = pool.tile([128, 1], f32, tag="s2c")
            nc.vector.tensor_scalar(
                s2c[:], sc[:], sc[:], 2.0, mybir.AluOpType.mult,
                mybir.AluOpType.mult,
            )
            czr = pool.tile([1, 1], f32, tag="czr")
            nc.vector.reciprocal(czr[:], ew[:, 128:129])
            ivc = pool.tile([128, 1], f32, tag="ivc")
            nc.vector.reciprocal(ivc[:], s2c[:])
            rsc = pool.tile([128, 1], f32, tag="rsc")
            nc.vector.reciprocal(rsc[:], sc[:])

            # ---- scale-multiplies ride the Scalar engine as Copy+scale ----
            t1h = pool.tile([1, 128], bf16, tag="t1h")
            nc.scalar.activation(
                t1h[:], ew[:, 0:128], mybir.ActivationFunctionType.Copy,
                scale=czr[:],
            )
            # transpose t1h (bf16); finish t1 = t1h / sigma on Scalar
            pTh = psum.tile([128, 1], bf16, tag="pTh")
            nc.tensor.matmul(pTh[:], t1h[:], oneb, is_transpose=True)
            expt = pool.tile([128, NTAP], bf16, tag="expt")
            nc.scalar.activation(
                expt[:], d2n[:], mybir.ActivationFunctionType.Exp, scale=ivc[:]
            )
            t1c = pool.tile([128, 1], bf16, tag="t1c")
            nc.scalar.activation(
                t1c[:], pTh[:], mybir.ActivationFunctionType.Copy, scale=rsc[:]
            )
            gp = psum.tile([64, 1], f32, tag="gp")
            nc.tensor.matmul(gp[:], expt[:], t1c[:], start=True, stop=True)
            # c = 1/sqrt(2 pi) folds into the gs copy's immediate scale
            gs = pool.tile([64, 1], bf16, tag="gs")
            nc.scalar.activation(
                gs[:], gp[:], mybir.ActivationFunctionType.Copy,
                scale=INV_SQRT_2PI,
            )

            # ---- binarize the Toeplitz mask (X-0.5 > 0), 2 DVE chunks ----
            mb = pool.tile([NTAP, TJ], bf16, tag="mb")
            for lo, hi in ((0, 512), (512, TJ)):
                nc.vector.tensor_scalar(
                    mb[:, lo:hi], mraw[:, lo:hi], 0.0, None, mybir.AluOpType.is_gt
                )

            # ---- output PSUM: input-independent zeroing matmul (runs at
            #      body start), then 4 concurrent col-tiled conv matmuls ----
            po = psum.tile([128, 256], f32, tag="po")
            nc.tensor.matmul(po[:], onesr[:], zrow[:], start=True, stop=True)
            for k in range(4):
                nc.tensor.matmul(
                    po[32 * k : 32 * k + 1, :], gs[:], mb[:, 256 * k : 256 * (k + 1)],
                    start=False, stop=False,
                    tile_position=(0, 32 * k),
                    skip_group_check=True,
                )

            # ---- add noise on the way out of PSUM, 4-descriptor store ----
            outS = pool.tile([128, 256], f32, tag="outS")
            nc.vector.tensor_add(outS[:], po[:], noiseS[:])
            out_src = AP(outS[:].tensor, outS[:].offset, [(32 * 256, 4), (1, 256)])
            nc.sync.dma_start(out_d[:], out_src)

    nc.compile()
    return nc


def kernel(X, sigma, weight, noise):
    global _compiled
    import ml_dtypes

    from concourse.bass_utils import run_bass_kernel_spmd

    X = np.ascontiguousarray(np.asarray(X, dtype=np.float32)).reshape(1, T)
    sigma = np.ascontiguousarray(np.asarray(sigma, dtype=np.float32)).reshape(E)
    weight = np.ascontiguousarray(np.asarray(weight, dtype=np.float32)).reshape(1, E)
    noise = np.ascontiguousarray(np.asarray(noise, dtype=np.float32)).reshape(1, T)

    if _compiled is None:
        _compiled = _build()
    nc = _compiled

    # mask window: 64*(X-0.5) as fp8 (sign-preserving affine shift; the
    # device binarizes with >0).  Window for core c covers global indices
    # [c*1024 - 32, c*1024 + 1055]; out-of-range pads to -32 (mask 0).
    Xpad = np.full(T + NTAP, -32.0, dtype=np.float32)
    Xpad[NTAP // 2 : NTAP // 2 + T] = 64.0 * (X[0] - 0.5)
    in_maps = []
    for c in range(N_CORES):
        crit = np.empty((1, 256), dtype=np.float32)
        crit[0, 0:128] = weight[0]
        crit[0, 128:256] = sigma
        maskr = Xpad[c * TJ : c * TJ + WIN].astype(ml_dtypes.float8_e4m3)
        nm = (
            noise[0, c * TJ : (c + 1) * TJ].reshape(4, 256).astype(ml_dtypes.bfloat16)
        )
        in_maps.append({"crit": crit, "maskr": maskr.reshape(1, WIN), "nm": nm})

    res = run_bass_kernel_spmd(nc, in_maps, core_ids=list(range(N_CORES)))
    out = np.empty((1, T), dtype=np.float32)
    for c in range(N_CORES):
        out[0, c * TJ : (c + 1) * TJ] = res.results[c]["out"].reshape(-1)
    return out


# revision 17
# speedup vs baseline: 1.0048x; 1.0048x over previous
"""Trainium2 Bass kernel for nn_CAGKE_1 (Gaussian-kernel embedding).

Math: reference computes, for mask m_i = 1[X_i > 0.5],
    out[j] = sum_e softmax(w)_e * sum_i m_i * (c/sigma_e) exp(-(j-i-1)^2/(2 sigma_e^2)) + noise_j
Both sums are linear, so the E=128 Gaussian channels collapse into one
combined kernel ghat(d) = sum_e softmax(w)_e * (c/sigma_e) exp(-d^2/(2 sigma_e^2))
BEFORE the convolution. sigma_max = 5 makes ghat(d) < 1e-9 for |d| >= 32,
so 64 taps (d = 31-u, u in [0,64)) cover it to far below the accuracy gate.

Layout strategy (per core, 1024 outputs):
  - The conv is ONE matmul shape: out[n] = sum_u gs[u] * M[u, n] with
    M[u, n] = m[base + u + n].  M is materialized by a single DMA whose
    DRAM access pattern has stride 1 in BOTH dims over a 1088-byte padded
    mask window - the 64x Toeplitz expansion happens inside the DMA
    (64 descriptors of 1KB), so the host ships 1.1KB instead of 144KB.
  - sigma/w arrive as ONE 256-wide f32 row on partition 0 (1 descriptor);
    the whole softmax/sigma chain runs in row space on partition 0, then
    two tiny PE transposes (moving = const [1,1] ones) lift
    t1 = c*exp(w)/(sigma*Z) and ivs = +1/(2 sigma^2) into column space.
    Z comes free from the Exp activation's accum_out, so there is no
    fp32 ones-matmul anywhere.  The exp table gets a NEGATED d^2 so ivs
    needs no sign flip.
  - The output lives on PSUM partitions {0,32,64,96} x 256 cols: four
    col-tiled matmuls (tile_position=(0,32k)) with the same 1-column gs
    stationary run CONCURRENTLY on the PE sub-arrays, each streaming a
    shifted 256-col slice of M.  A sparse-stationary matmul (S4[k,32k]=1)
    pre-loads the noise into the same PSUM region (start=True), so the
    epilogue is one DVE copy + a 4-descriptor store.
  - The X-0.5 shift is scaled by 64 on the host (pure affine + fp8 cast,
    as in the reference's binarize-vs-0.5) so the sign survives fp8;
    the device binarizes with is_gt 0.
"""

import sys

import numpy as np

if "/opt/trn_rl_repo" not in sys.path:
    sys.path.insert(0, "/opt/trn_rl_repo")

T = 8192
E = 128
N_CORES = 8
TJ = T // N_CORES          # 1024 outputs per core
NTAP = 64                  # ghat taps: d = 31 - u, u in [0, 64)
WIN = NTAP + TJ            # 1088-byte padded mask window per core
INV_SQRT_2PI = 0.39894228

_compiled = None


def _build():
    import concourse.bacc as bacc
    import concourse.bass as bass
    import concourse.mybir as mybir
    import concourse.tile as tile
    from concourse.ap import AP

    f32 = mybir.dt.float32
    bf16 = mybir.dt.bfloat16
    fp8 = mybir.dt.float8e4
    nc = bacc.Bacc(num_devices=N_CORES, debug=False)

    crit_d = nc.dram_tensor("crit", [1, 256], f32, kind="ExternalInput")
    maskr_d = nc.dram_tensor("maskr", [1, WIN], fp8, kind="ExternalInput")
    nm_d = nc.dram_tensor("nm", [4, 256], bf16, kind="ExternalInput")
    out_d = nc.dram_tensor("out", [4, 256], f32, kind="ExternalOutput")

    with tile.TileContext(nc) as tc:
        with (
            tc.tile_pool(name="pool", bufs=1) as pool,
            tc.tile_pool(name="psum", bufs=1, space="PSUM") as psum,
        ):
            # ---- three HWDGE loads.  The mask rides the Scalar engine's
            # HWDGE ring (issued before the ACT table load, whose 1.3us
            # overlaps the flight); sigma/w + noise ride Sync. ----
            mraw = pool.tile([NTAP, TJ], fp8, tag="mraw")
            mask_src = AP(maskr_d[:].tensor, 0, [(1, NTAP), (1, TJ)])
            nc.scalar.dma_start(mraw[:], mask_src)
            crit = pool.tile([1, 256], f32, tag="crit")
            nc.sync.dma_start(crit[:], crit_d[:])
            # noise straight to SBUF partitions {0,32,64,96} (4 descriptors);
            # dead partitions are zeroed early by GpSimd so the final
            # full-partition add reads defined data
            noiseS = pool.tile([128, 256], bf16, tag="noiseS")
            nc.gpsimd.memset(noiseS[:], 0.0)
            noise_dst = AP(
                noiseS[:].tensor, noiseS[:].offset, [(32 * 256, 4), (1, 256)]
            )
            nc.sync.dma_start(noise_dst, nm_d[:])

            wrow = crit[0:1, 0:128]
            srow = crit[0:1, 128:256]
            onef = nc.const_aps.tensor(1.0, (1, 1), f32)
            oneb = nc.const_aps.tensor(1.0, (1, 1), bf16)

            # ---- input-independent prep (Pool engine, off-path) ----
            # d2n[., u] = -(u-31)^2 via (u-31)*(31-u)
            dlt = pool.tile([128, NTAP], f32, tag="dlt")
            nc.gpsimd.iota(
                dlt[:], pattern=[[1, NTAP]], base=-31, channel_multiplier=0,
                allow_small_or_imprecise_dtypes=True,
            )
            dltn = pool.tile([128, NTAP], f32, tag="dltn")
            nc.gpsimd.iota(
                dltn[:], pattern=[[-1, NTAP]], base=31, channel_multiplier=0,
                allow_small_or_imprecise_dtypes=True,
            )
            d2n = pool.tile([128, NTAP], f32, tag="d2n")
            nc.gpsimd.tensor_mul(d2n[:], dlt[:], dltn[:])
            onesr = pool.tile([1, 128], bf16, tag="onesr")
            nc.gpsimd.memset(onesr[:], 1.0)
            zrow = pool.tile([1, 256], bf16, tag="zrow")
            nc.gpsimd.memset(zrow[:], 0.0)

            # ---- transpose sigma early: row -> column (PE, f32) ----
            pSc = psum.tile([128, 1], f32, tag="pSc")
            nc.tensor.matmul(pSc[:], srow, onef, is_transpose=True)
            # softmax exp in row space; accum_out gives Z for free
            ew = pool.tile([1, 132], f32, tag="ew")
            nc.scalar.activation(
                ew[:, 0:128], wrow, mybir.ActivationFunctionType.Exp,
                accum_out=ew[:, 128:129],
            )

            # ---- binarize chunk 1 first on the DVE queue (mask lands
            #      before sigma/w), then the critical column chain ----
            mb = pool.tile([NTAP, TJ], bf16, tag="mb")
            nc.vector.tensor_scalar(
                mb[:, 0:512], mraw[:, 0:512], 0.0, None, mybir.AluOpType.is_gt
            )
            sc = pool.tile([128, 1], f32, tag="sc")
            nc.vector.tensor_copy(sc[:], pSc[:])
            # 2*sigma^2 on GpSimd (SBUF-only op, Pool queue is idle)
            s2c = pool.tile([128, 1], f32, tag="s2c")
            nc.gpsimd.tensor_scalar(
                s2c[:], sc[:], sc[:], 2.0, mybir.AluOpType.mult,
                mybir.AluOpType.mult,
            )
            czr = pool.tile([1, 1], f32, tag="czr")
            nc.vector.reciprocal(czr[:], ew[:, 128:129])
            ivc = pool.tile([128, 1], f32, tag="ivc")
            nc.vector.reciprocal(ivc[:], s2c[:])
            rsc = pool.tile([128, 1], f32, tag="rsc")
            nc.vector.reciprocal(rsc[:], sc[:])
            # t1h = exp(w) * c/Z in row space (c folds into the dual-op)
            t1h = pool.tile([1, 128], bf16, tag="t1h")
            nc.vector.tensor_scalar(
                t1h[:], ew[:, 0:128], czr[:], INV_SQRT_2PI,
                mybir.AluOpType.mult, mybir.AluOpType.mult,
            )
            # binarize chunk 2, data-gated behind the critical chain via a
            # derived zero threshold so the static schedule cannot hoist it
            thr = pool.tile([64, 1], f32, tag="thr")
            nc.gpsimd.tensor_scalar_mul(thr[:], ivc[0:64, :], 0.0)
            nc.vector.tensor_scalar(
                mb[:, 512:TJ], mraw[:, 512:TJ], thr[:], None, mybir.AluOpType.is_gt
            )

            # transpose t1h (bf16); finish t1 = t1h / sigma
            pTh = psum.tile([128, 1], bf16, tag="pTh")
            nc.tensor.matmul(pTh[:], t1h[:], oneb, is_transpose=True)
            expt = pool.tile([128, NTAP], bf16, tag="expt")
            nc.scalar.activation(
                expt[:], d2n[:], mybir.ActivationFunctionType.Exp, scale=ivc[:]
            )
            t1c = pool.tile([128, 1], bf16, tag="t1c")
            nc.vector.tensor_mul(t1c[:], pTh[:], rsc[:])
            gp = psum.tile([64, 1], f32, tag="gp")
            nc.tensor.matmul(gp[:], expt[:], t1c[:], start=True, stop=True)
            gs = pool.tile([64, 1], bf16, tag="gs")
            nc.vector.tensor_copy(gs[:], gp[:])

            # ---- output PSUM: input-independent zeroing matmul (runs at
            #      body start), then 4 concurrent col-tiled conv matmuls ----
            po = psum.tile([128, 256], f32, tag="po")
            nc.tensor.matmul(po[:], onesr[:], zrow[:], start=True, stop=True)
            for k in range(4):
                nc.tensor.matmul(
                    po[32 * k : 32 * k + 1, :], gs[:], mb[:, 256 * k : 256 * (k + 1)],
                    start=False, stop=False,
                    tile_position=(0, 32 * k),
                    skip_group_check=True,
                )

            # ---- add noise on the way out of PSUM, 4-descriptor store ----
            outS = pool.tile([128, 256], f32, tag="outS")
            nc.vector.tensor_add(outS[:], po[:], noiseS[:])
            out_src = AP(outS[:].tensor, outS[:].offset, [(32 * 256, 4), (1, 256)])
            nc.sync.dma_start(out_d[:], out_src)

    nc.compile()
    return nc


def kernel(X, sigma, weight, noise):
    global _compiled
    import ml_dtypes

    from concourse.bass_utils import run_bass_kernel_spmd

    X = np.ascontiguousarray(np.asarray(X, dtype=np.float32)).reshape(1, T)
    sigma = np.ascontiguousarray(np.asarray(sigma, dtype=np.float32)).reshape(E)
    weight = np.ascontiguousarray(np.asarray(weight, dtype=np.float32)).reshape(1, E)
    noise = np.ascontiguousarray(np.asarray(noise, dtype=np.float32)).reshape(1, T)

    if _compiled is None:
        _compiled = _build()
    nc = _compiled

    # mask window: 64*(X-0.5) as fp8 (sign-preserving affine shift; the
    # device binarizes with >0).  Window for core c covers global indices
    # [c*1024 - 32, c*1024 + 1055]; out-of-range pads to -32 (mask 0).
    Xpad = np.full(T + NTAP, -32.0, dtype=np.float32)
    Xpad[NTAP // 2 : NTAP // 2 + T] = 64.0 * (X[0] - 0.5)
    in_maps = []
    for c in range(N_CORES):
        crit = np.empty((1, 256), dtype=np.float32)
        crit[0, 0:128] = weight[0]
        crit[0, 128:256] = sigma
        maskr = Xpad[c * TJ : c * TJ + WIN].astype(ml_dtypes.float8_e4m3)
        nm = (
            noise[0, c * TJ : (c + 1) * TJ].reshape(4, 256).astype(ml_dtypes.bfloat16)
        )
        in_maps.append({"crit": crit, "maskr": maskr.reshape(1, WIN), "nm": nm})

    res = run_bass_kernel_spmd(nc, in_maps, core_ids=list(range(N_CORES)))
    out = np.empty((1, T), dtype=np.float32)
    for c in range(N_CORES):
        out[0, c * TJ : (c + 1) * TJ] = res.results[c]["out"].reshape(-1)
    return out
